# revision 6
# baseline (speedup 1.0000x reference)
"""Trainium2 Bass kernel for nn_Explainer (gnn_message_passing).

Math (reference):
  f12[i*n+j] = concat(embed[i], embed[j]);  h = relu(f12 @ W1 + b1)
  log_alpha = h @ W2 + b2
  gate = sigmoid((log(u) - log(1-u) + log_alpha) / beta)
  sym = (gate + gate.T)/2 ; masked = adj * sym
  hg = relu((masked @ x) @ Wg1); pooled = hg.mean(0); softmax(pooled @ Wg2)

Key decomposition: f12 @ W1 + b1 = A[i] + B[j] with
  A = embed @ W1[:64] + b1   (per-row), B = embed @ W1[64:]
so log_alpha[i,j] = W2 . relu(A[i] + B[j]) + b2 -- no [N^2,128] matmul needed.

Sharding: row-blocks of the i dimension across 8 cores. The gate matrix
column-block needed for symmetrization is exchanged with AllToAlls. The
edge-MLP psum accumulates in 4 row-groups of 32; as each group finishes,
its gate rows are computed and AllToAll'd immediately, pipelining the
exchange under the remaining edge-MLP compute. Per-core class logits are
computed locally and combined with a tiny AllReduce; softmax is replicated.
"""
import numpy as np

import concourse.bass as bass
import concourse.bacc as bacc
import concourse.tile as tile
from concourse import mybir
from concourse.bass_utils import run_bass_kernel_spmd

N = 1024
NC = 8
R = N // NC          # 128 rows per core
D = 64               # embed dim
H = 64               # hidden
F = 128              # x features
C = 8                # classes
NPAIR = R // 2       # 64 i-pairs per core
GRP = 16             # pairs per PE column-group / psum row-group
NCHUNK = NPAIR // GRP  # 4 row-chunks of 32 gate rows
CH = 2 * GRP         # 32 gate rows per chunk

F32 = mybir.dt.float32
BF16 = mybir.dt.bfloat16

# dtype used for the edge-MLP reduction matmul stream (PE runs 4x slower on f32)
MM_DT = BF16
DEBUG_OUTPUTS = False


def _mask_w2_np():
    """[128, NPAIR, 32] mask: 1.0 where the block-diag W2 stack has W2 values.

    Pair t -> psum row-group g=t//16 (tile_position=(0,32g)), slot s=t%16.
    lhsT_t = W2S[:, t, :]: col 2s rows 0:64 = W2, col 2s+1 rows 64:128 = W2.
    psum out row for pair t = 32g + 2s (+1) = 2t (+1) = local i'.
    """
    cols = 32
    m = np.zeros((128, NPAIR, cols), np.float32)
    for t in range(NPAIR):
        s = t % GRP
        m[0:64, t, 2 * s] = 1.0
        m[64:128, t, 2 * s + 1] = 1.0
    return m


def build():
    nc = bacc.Bacc("TRN2", target_bir_lowering=False, debug=False, num_devices=NC)

    # ---- kernel I/O ----
    # embT_full = embed.T (layout prep on host); embT_slab = embed[block].T
    # adjcol_slab = adj[:, block]  (column slab, row-major)
    embT_in = nc.dram_tensor("embT_in", [D, N], F32, kind="ExternalInput")
    embTs_in = nc.dram_tensor("embTs_in", [D, R], F32, kind="ExternalInput")
    x_full = nc.dram_tensor("x_full", [N, F], F32, kind="ExternalInput")
    adjcol_in = nc.dram_tensor("adjcol_in", [N, R], F32, kind="ExternalInput")
    noise_slab = nc.dram_tensor("noise_slab", [R, N], F32, kind="ExternalInput")
    tmp_in = nc.dram_tensor("tmp_in", [1, 1], F32, kind="ExternalInput")
    w1_in = nc.dram_tensor("w1_in", [2 * D, H], F32, kind="ExternalInput")
    b1_in = nc.dram_tensor("b1_in", [1, H], F32, kind="ExternalInput")
    w2_in = nc.dram_tensor("w2_in", [H, 1], F32, kind="ExternalInput")
    b2_in = nc.dram_tensor("b2_in", [1, 1], F32, kind="ExternalInput")
    wg1_in = nc.dram_tensor("wg1_in", [F, H], F32, kind="ExternalInput")
    wg2_in = nc.dram_tensor("wg2_in", [H, C], F32, kind="ExternalInput")
    out_dram = nc.dram_tensor("out", [1, C], F32, kind="ExternalOutput")
    dbg = {}
    if DEBUG_OUTPUTS:
        for nm, shp in [("d_la", [R, N]), ("d_gate", [R, N]),
                        ("d_mh", [128, N]), ("d_tT", [128, 128]),
                        ("d_pooled", [1, H]), ("d_lgp", [1, C])]:
            dbg[nm] = nc.dram_tensor(nm, shp, F32, kind="ExternalOutput")

    # ---- compile-time constants ----
    identb_c = nc.inline_tensor(np.eye(128, dtype=np.float32).astype(
        mybir.dt.np(BF16)), name="identb")
    maskw2_c = nc.inline_tensor(_mask_w2_np().astype(
        np.float32).astype(mybir.dt.np(MM_DT)), name="maskw2")
    ones128_c = nc.inline_tensor(np.ones((1, 128), np.float32), name="ones128")
    syncsrc_c = nc.inline_tensor(np.zeros((1, 8), np.float32), name="syncsrc")

    with tile.TileContext(nc) as tc:
        with (
            tc.tile_pool(name="const", bufs=1) as constp,
            tc.tile_pool(name="big", bufs=1) as big,
            tc.tile_pool(name="tmpp", bufs=6) as tmpp,
            tc.tile_pool(name="pla", bufs=1, space="PSUM") as pla,
            tc.tile_pool(name="ptp", bufs=2, space="PSUM") as ptp,
            tc.tile_pool(name="psm", bufs=2, space="PSUM") as psm,
            tc.tile_pool(name="dram", bufs=1, space="DRAM") as dram,
        ):
            # ============ phase 0: early sync + loads + precompute ==========
            # Early rendezvous on the CC queue: doorbell depends only on an
            # inline const, so it rings ~1us in and absorbs the first-
            # collective fixed cost + PJRT launch skew under phase 1.
            sync_out = dram.tile([NC, 8], F32, addr_space="Shared")
            nc.gpsimd.collective_compute(
                "AllGather", mybir.AluOpType.bypass,
                replica_groups=[list(range(NC))],
                ins=[syncsrc_c[:].opt()], outs=[sync_out[:].opt()])

            # critical-path loads first, all on the sync (SP) queue: its DMA
            # dispatch does not steal time from DVE/ACT which run the relu.
            embT = big.tile([D, N], F32)
            nc.sync.dma_start(embT[:], embT_in[:])
            w1a_sb = big.tile([D, H], F32)
            nc.sync.dma_start(w1a_sb[:], w1_in[0:D, :])
            w1b_sb = big.tile([D, H], F32)
            nc.sync.dma_start(w1b_sb[:], w1_in[D:2 * D, :])
            eTs = big.tile([D, R], F32)
            nc.sync.dma_start(eTs[:], embTs_in[:])
            b1t_sb = big.tile([H, 1], F32)
            nc.sync.dma_start(b1t_sb[:], b1_in[:].rearrange("o h -> h o"))
            w2_sb = big.tile([H, 1], F32)
            nc.sync.dma_start(w2_sb[:], w2_in[:])
            maskw2 = constp.tile([128, NPAIR, 32], MM_DT)
            nc.sync.dma_start(maskw2[:, 0:GRP, :], maskw2_c[:, 0:GRP, :])
            noise_sb = big.tile([R, N], F32)
            nc.sync.dma_start(noise_sb[:], noise_slab[:])
            for g in range(1, NCHUNK):
                nc.sync.dma_start(maskw2[:, g * GRP:(g + 1) * GRP, :],
                                  maskw2_c[:, g * GRP:(g + 1) * GRP, :])
            x_sb = big.tile([128, NC, F], F32)
            nc.sync.dma_start(
                x_sb[:], x_full[:].rearrange("(r p) f -> p r f", p=128))
            adjT = big.tile([128, NC, 128], F32)
            nc.sync.dma_start(
                adjT[:], adjcol_in[:].rearrange("(r p) b -> p r b", p=128))

            # small/late loads on the gpsimd (Pool) queue (cheap dispatch)
            identb = constp.tile([128, 128], BF16)
            nc.gpsimd.dma_start(identb[:], identb_c[:])
            ones128 = constp.tile([1, 128], F32)
            nc.gpsimd.dma_start(ones128[:], ones128_c[:])
            b2_sb = big.tile([1, 1], F32)
            nc.gpsimd.dma_start(b2_sb[:], b2_in[:])
            tmp_sb = big.tile([1, 1], F32)
            nc.gpsimd.dma_start(tmp_sb[:], tmp_in[:])
            wg1_sb = big.tile([F, H], F32)
            nc.gpsimd.dma_start(wg1_sb[:], wg1_in[:])
            wg2_sb = big.tile([H, C], F32)
            nc.gpsimd.dma_start(wg2_sb[:], wg2_in[:])

            # PE warm-up: dependency-free dummy matmuls so the HAM clock-gate
            # opens (1.2 -> 2.4 GHz) before the real matmul stream starts.
            warm_sb = tmpp.tile([128, 256], MM_DT, tag="warm")
            nc.vector.memset(warm_sb[:], 0.0)
            for _ in range(10):
                warm_ps = psm.tile([1, 256], F32, tag="sm", name="warm_ps")
                nc.tensor.matmul(warm_ps[:], warm_sb[:, 0:1], warm_sb[:])

            # bf16 copies of the GNN operands on gpsimd (idle during phase 1)
            x_bf = big.tile([128, NC, F], BF16)
            nc.gpsimd.tensor_copy(x_bf[:], x_sb[:])
            adjT_bf = big.tile([128, NC, 128], BF16)
            nc.gpsimd.tensor_copy(adjT_bf[:], adjT[:])

            # scaled GNN weights: 0.5 into Wg1 (symmetrize), 1/1024 into Wg2
            # (mean); Wg1 in bf16 for the hg matmul.
            wg1h = big.tile([F, H], BF16)
            nc.gpsimd.tensor_scalar(out=wg1h[:], in0=wg1_sb[:],
                                    scalar1=0.5, scalar2=None,
                                    op0=mybir.AluOpType.mult)
            wg2s = big.tile([H, C], F32)
            nc.gpsimd.tensor_scalar(out=wg2s[:], in0=wg2_sb[:],
                                    scalar1=1.0 / N, scalar2=None,
                                    op0=mybir.AluOpType.mult)

            # A^T for this core's slab: [64, 128] = W1a^T @ embed_slab^T + b1
            at_ps = psm.tile([H, R], F32, tag="sm")
            nc.tensor.matmul(at_ps[:], w1a_sb[:], eTs[:])
            ats = big.tile([H, R], F32)
            nc.vector.tensor_scalar(out=ats[:], in0=at_ps[:],
                                    scalar1=b1t_sb[:], scalar2=None,
                                    op0=mybir.AluOpType.add)
            # ATstack [128, 64]: col t = bias column for pair t
            atstack = big.tile([128, NPAIR], F32)
            ats_pair = ats[:].rearrange("h (t two) -> h two t", two=2)
            nc.vector.tensor_copy(atstack[0:H, :], ats_pair[:, 0, :])
            nc.vector.tensor_copy(atstack[H:128, :], ats_pair[:, 1, :])

            # B^T (full): [64, 1024], then stacked twice -> [128, 1024] bf16
            btstack = big.tile([128, N], MM_DT)
            for jc in range(2):
                bt_ps = psm.tile([H, 512], F32, tag="sm")
                nc.tensor.matmul(bt_ps[:], w1b_sb[:],
                                 embT[:, jc * 512:(jc + 1) * 512])
                nc.vector.tensor_copy(
                    btstack[0:H, jc * 512:(jc + 1) * 512], bt_ps[:])
                nc.scalar.copy(
                    btstack[H:128, jc * 512:(jc + 1) * 512], bt_ps[:])

            # W2 stacks: maskw2 * [W2; W2] per-partition, built per group so
            # group 0 is ready as soon as its maskw2 slice lands
            w2col = big.tile([128, 1], F32)
            nc.vector.tensor_copy(w2col[0:H, :], w2_sb[:])
            nc.vector.tensor_copy(w2col[H:128, :], w2_sb[:])
            w2s_t = big.tile([128, NPAIR, 32], MM_DT)
            for g in range(NCHUNK):
                nc.vector.tensor_scalar(
                    out=w2s_t[:, g * GRP:(g + 1) * GRP, :].rearrange(
                        "p t c -> p (t c)"),
                    in0=maskw2[:, g * GRP:(g + 1) * GRP, :].rearrange(
                        "p t c -> p (t c)"),
                    scalar1=w2col[:], scalar2=None,
                    op0=mybir.AluOpType.mult)

            # gate scale/bias: sigmoid(invb * pre + invb*b2)
            invb = big.tile([1, 1], F32)
            nc.vector.reciprocal(invb[:], tmp_sb[:])
            ib2 = big.tile([1, 1], F32)
            nc.vector.tensor_tensor(ib2[:], invb[:], b2_sb[:],
                                    op=mybir.AluOpType.mult)
            invb_ps = psm.tile([128, 1], F32, tag="sm")
            nc.tensor.matmul(invb_ps[:], ones128[:], invb[:])
            invb128 = big.tile([128, 1], F32)
            nc.vector.tensor_copy(invb128[:], invb_ps[:])
            ib2_ps = psm.tile([128, 1], F32, tag="sm")
            nc.tensor.matmul(ib2_ps[:], ones128[:], ib2[:])
            ib2b = big.tile([128, 1], F32)
            nc.vector.tensor_copy(ib2b[:], ib2_ps[:])

            # noise transform on ACT before its relu share: nl = ln(u)-ln(1-u)
            logu = big.tile([R, N], F32)
            nc.scalar.activation(logu[:], noise_sb[:],
                                 mybir.ActivationFunctionType.Ln)
            log1mu = big.tile([R, N], F32)
            nc.scalar.activation(log1mu[:], noise_sb[:],
                                 mybir.ActivationFunctionType.Ln,
                                 bias=1.0, scale=-1.0)
            nl = big.tile([R, N], F32)
            nc.vector.tensor_tensor(nl[:], logu[:], log1mu[:],
                                    op=mybir.AluOpType.subtract)

            # ========= phase 1: edge MLP with pipelined gate exchange =======
            # la[2t + a, j] = sum_k W2[k] relu(A[2t+a, k] + B[j, k]).
            # psum row-group g (rows 32g..32g+32) completes at pair t=16g+15;
            # its 32 gate rows are then computed and AllToAll'd immediately.
            la_ps = [pla.tile([128, 512], F32, tag=f"la{jc}", name=f"la_ps{jc}")
                     for jc in range(2)]
            gate = big.tile([R, N], BF16)
            pre = big.tile([R, N], F32)
            a2a_in = [dram.tile([NC * CH, 128], BF16, name=f"a2a_in{g}")
                      for g in range(NCHUNK)]
            a2a_out = [dram.tile([NC * CH, 128], BF16, name=f"a2a_out{g}")
                       for g in range(NCHUNK)]
            for t in range(NPAIR):
                g, s = t // GRP, t % GRP
                tmpb = tmpp.tile([128, N], MM_DT, tag="relu")
                if t % 5 in (2, 4):
                    nc.scalar.activation(
                        tmpb[:], btstack[:],
                        mybir.ActivationFunctionType.Relu,
                        bias=atstack[:, t:t + 1])
                else:
                    nc.vector.tensor_scalar(
                        out=tmpb[:], in0=btstack[:],
                        scalar1=atstack[:, t:t + 1], scalar2=0.0,
                        op0=mybir.AluOpType.add, op1=mybir.AluOpType.max)
                for jc in range(2):
                    nc.tensor.matmul(
                        la_ps[jc][32 * g:32 * (g + 1), :],
                        w2s_t[:, t, :],
                        tmpb[:, jc * 512:(jc + 1) * 512],
                        start=(s == 0), stop=(s == GRP - 1),
                        tile_position=(0, 32 * g))
                if s == GRP - 1:
                    # group g's psum rows are final: gate chunk + stage + A2A
                    lo, hi = 32 * g, 32 * (g + 1)
                    for jc in range(2):
                        nc.vector.tensor_tensor(
                            pre[lo:hi, jc * 512:(jc + 1) * 512],
                            la_ps[jc][lo:hi, :],
                            nl[lo:hi, jc * 512:(jc + 1) * 512],
                            op=mybir.AluOpType.add)
                    nc.scalar.activation(
                        gate[lo:hi, :], pre[lo:hi, :],
                        mybir.ActivationFunctionType.Sigmoid,
                        bias=ib2b[lo:hi, :], scale=invb128[lo:hi, :])
                    # stage: a2a_in[g][(r m), i] = gate[lo+m, 128r+i]
                    nc.sync.dma_start(
                        a2a_in[g][:].rearrange("(r m) i -> m r i", r=NC),
                        gate[lo:hi, :].rearrange("m (r i) -> m r i", r=NC))

            # ACT exp-table preload: runs during the A2A/AllReduce waits so
            # the final softmax exp does not pay the 1.3us table switch.
            # (relu lives in every table, so later relus stay cheap.)
            dexp = big.tile([1, 8], F32)
            nc.scalar.activation(dexp[:], ones128[:, 0:8],
                                 mybir.ActivationFunctionType.Exp)

            # collective doorbells + output scatters on gpsimd, interleaved
            # in expected completion order
            gcolT = big.tile([128, NC, 128], BF16)

            def ring(g):
                nc.gpsimd.collective_compute(
                    "AllToAll", mybir.AluOpType.bypass,
                    replica_groups=[list(range(NC))],
                    ins=[a2a_in[g][:].opt()], outs=[a2a_out[g][:].opt()])

            def scatter(g):
                # gcolT[32g+m, r, i] = gate[128r+32g+m (global row), own col i]
                nc.gpsimd.dma_start(
                    gcolT[32 * g:32 * (g + 1), :, :],
                    a2a_out[g][:].rearrange("(r m) i -> m r i", r=NC))

            ring(0)
            ring(1)
            scatter(0)
            ring(2)
            scatter(1)
            ring(3)
            scatter(2)
            scatter(3)

            # own slab transposed: gT[jl, r, i'] = gate[i', 128r+jl]
            gTc = big.tile([128, NC, 128], BF16)
            for r in range(NC):
                pt = ptp.tile([128, 128], BF16, tag="tp")
                nc.tensor.transpose(pt[:], gate[:, r * 128:(r + 1) * 128],
                                    identb[:])
                nc.vector.tensor_copy(gTc[:, r, :], pt[:])

            # ================= phase 3: mask + GNN ==========================
            # masked^T = (gT + gcolT) * adjT  (0.5 folded into Wg1), all bf16
            msum = big.tile([128, N], BF16)
            nc.vector.tensor_tensor(msum[:],
                                    gTc[:].rearrange("p r b -> p (r b)"),
                                    gcolT[:].rearrange("p r b -> p (r b)"),
                                    op=mybir.AluOpType.add)
            mh_bf = big.tile([128, N], BF16)
            nc.vector.tensor_tensor(mh_bf[:], msum[:],
                                    adjT_bf[:].rearrange("p r b -> p (r b)"),
                                    op=mybir.AluOpType.mult)

            # tT[f, i'] = sum_j x[j, f] masked[i', j]
            tT_ps = pla.tile([128, 128], F32, tag="tT")
            for r in range(NC):
                nc.tensor.matmul(
                    tT_ps[:], x_bf[:, r, :],
                    mh_bf[:, r * 128:(r + 1) * 128],
                    start=(r == 0), stop=(r == NC - 1))
            tT = big.tile([128, 128], BF16)
            nc.vector.tensor_copy(tT[:], tT_ps[:])

            # hgT = relu(Wg1h^T @ tT): [64, 128]; pooled partial via ACT accum
            hg_ps = psm.tile([H, 128], F32, tag="sm")
            nc.tensor.matmul(hg_ps[:], wg1h[:], tT[:])
            hgT = big.tile([H, 128], F32)
            pooled = big.tile([H, 1], F32)
            nc.scalar.activation(hgT[:], hg_ps[:],
                                 mybir.ActivationFunctionType.Relu,
                                 accum_out=pooled[:])

            # local class logits, then AllReduce-add the [1, C] across cores
            lg_ps = psm.tile([1, C], F32, tag="sm")
            nc.tensor.matmul(lg_ps[:], pooled[:], wg2s[:])
            lgp = big.tile([1, C], F32)
            nc.vector.tensor_copy(lgp[:], lg_ps[:])
            ar_in = dram.tile([1, C], F32)
            nc.gpsimd.dma_start(ar_in[:], lgp[:])
            ar_out = dram.tile([1, C], F32, addr_space="Shared")
            nc.gpsimd.collective_compute(
                "AllReduce", mybir.AluOpType.add,
                replica_groups=[list(range(NC))],
                ins=[ar_in[:].opt()], outs=[ar_out[:].opt()])
            z = big.tile([1, C], F32)
            nc.sync.dma_start(z[:], ar_out[:])

            # softmax on [1, 8] (logits are O(1): skip the max-subtraction)
            e = big.tile([1, C], F32)
            ssum = big.tile([1, 1], F32)
            nc.scalar.activation(e[:], z[:],
                                 mybir.ActivationFunctionType.Exp,
                                 accum_out=ssum[:])
            rinv = big.tile([1, 1], F32)
            nc.vector.reciprocal(rinv[:], ssum[:])
            sm = big.tile([1, C], F32)
            nc.vector.tensor_scalar(out=sm[:], in0=e[:], scalar1=rinv[:],
                                    scalar2=None, op0=mybir.AluOpType.mult)
            nc.sync.dma_start(out_dram[:], sm[:])

            if DEBUG_OUTPUTS:
                laf = big.tile([R, N], F32)
                for jc in range(2):
                    nc.vector.tensor_tensor(
                        laf[:, jc * 512:(jc + 1) * 512], pre[:, jc * 512:(jc + 1) * 512],
                        nl[:, jc * 512:(jc + 1) * 512], op=mybir.AluOpType.subtract)
                nc.sync.dma_start(dbg["d_la"][:], laf[:])
                gf = big.tile([R, N], F32)
                nc.vector.tensor_copy(gf[:], gate[:])
                nc.sync.dma_start(dbg["d_gate"][:], gf[:])
                mf = big.tile([128, N], F32)
                nc.vector.tensor_copy(mf[:], mh_bf[:])
                nc.sync.dma_start(dbg["d_mh"][:], mf[:])
                tf = big.tile([128, 128], F32)
                nc.vector.tensor_copy(tf[:], tT[:])
                nc.sync.dma_start(dbg["d_tT"][:], tf[:])
                nc.sync.dma_start(dbg["d_pooled"][:].rearrange("o h -> h o"),
                                  pooled[:])
                nc.sync.dma_start(dbg["d_lgp"][:], lgp[:])

    nc.compile()
    return nc


_NC_CACHE = None
_RUNNER_CACHE = None


def _get_nc():
    global _NC_CACHE
    if _NC_CACHE is None:
        _NC_CACHE = build()
    return _NC_CACHE


def _get_runner():
    """Cached jitted 8-core executable (run_bass_via_pjrt rebuilds the jit
    wrapper every call, costing ~300ms of host time per invocation)."""
    global _RUNNER_CACHE
    if _RUNNER_CACHE is not None:
        return _RUNNER_CACHE
    import jax
    from jax.sharding import Mesh, PartitionSpec
    from jax.experimental.shard_map import shard_map
    from concourse import mybir as mb
    from concourse.bass2jax import (_bass_exec_p, install_neuronx_cc_hook,
                                    partition_id_tensor)

    nc = _get_nc()
    install_neuronx_cc_hook()
    partition_name = (nc.partition_id_tensor.name
                      if nc.partition_id_tensor else None)
    in_names, out_names, out_avals, zero_outs = [], [], [], []
    for alloc in nc.m.functions[0].allocations:
        if not isinstance(alloc, mb.MemoryLocationSet):
            continue
        name = alloc.memorylocations[0].name
        if alloc.kind == "ExternalInput":
            if name == partition_name:
                continue
            in_names.append(name)
        elif alloc.kind == "ExternalOutput":
            shape = tuple(alloc.tensor_shape)
            dtype = mb.dt.np(alloc.dtype)
            out_names.append(name)
            out_avals.append(jax.core.ShapedArray(shape, dtype))
            zero_outs.append(np.zeros(shape, dtype))
    n_params = len(in_names)
    all_in = in_names + out_names
    if partition_name is not None:
        all_in = all_in + [partition_name]

    def _body(*args):
        operands = list(args)
        if partition_name is not None:
            operands.append(partition_id_tensor())
        outs = _bass_exec_p.bind(
            *operands,
            out_avals=tuple(out_avals),
            in_names=tuple(all_in),
            out_names=tuple(out_names),
            lowering_input_output_aliases=(),
            sim_require_finite=True,
            sim_require_nnan=True,
            nc=nc,
        )
        return tuple(outs)

    devices = jax.devices()[:NC]
    mesh = Mesh(np.asarray(devices), ("core",))
    n_outs = len(out_names)
    sharded = jax.jit(
        shard_map(_body, mesh=mesh,
                  in_specs=(PartitionSpec("core"),) * (n_params + n_outs),
                  out_specs=(PartitionSpec("core"),) * n_outs,
                  check_rep=False),
        donate_argnums=tuple(range(n_params, n_params + n_outs)),
        keep_unused=True)

    def run(in_maps):
        concat_in = [
            np.concatenate([np.asarray(in_maps[c][nm]) for c in range(NC)],
                           axis=0)
            for nm in in_names
        ]
        concat_zeros = [
            np.zeros((NC * z.shape[0], *z.shape[1:]), z.dtype)
            for z in zero_outs
        ]
        out_arrs = sharded(*concat_in, *concat_zeros)
        return [
            {nm: np.asarray(out_arrs[i]).reshape(NC, *out_avals[i].shape)[c]
             for i, nm in enumerate(out_names)}
            for c in range(NC)
        ]

    _RUNNER_CACHE = run
    return run


def kernel(**inputs):
    x = np.ascontiguousarray(np.asarray(inputs["x"], dtype=np.float32))
    embed = np.ascontiguousarray(np.asarray(inputs["embed"], dtype=np.float32))
    adj = np.ascontiguousarray(np.asarray(inputs["adj"], dtype=np.float32))
    tmp = np.asarray(inputs["tmp"], dtype=np.float32).reshape(1, 1)
    noise = np.asarray(inputs["noise"], dtype=np.float32).reshape(N, N)
    W1 = np.ascontiguousarray(np.asarray(inputs["W1"], dtype=np.float32))
    b1 = np.asarray(inputs["b1"], dtype=np.float32).reshape(1, H)
    W2 = np.ascontiguousarray(np.asarray(inputs["W2"], dtype=np.float32))
    b2 = np.asarray(inputs["b2"], dtype=np.float32).reshape(1, 1)
    Wg1 = np.ascontiguousarray(np.asarray(inputs["Wg1"], dtype=np.float32))
    Wg2 = np.ascontiguousarray(np.asarray(inputs["Wg2"], dtype=np.float32))

    in_maps = build_in_maps(x, embed, adj, noise, tmp, W1, b1, W2, b2, Wg1, Wg2)
    try:
        results = _get_runner()(in_maps)
        return np.asarray(results[0]["out"], dtype=np.float32).reshape(1, C)
    except Exception:
        nc = _get_nc()
        res = run_bass_kernel_spmd(nc, in_maps, core_ids=list(range(NC)))
        return np.asarray(res.results[0]["out"],
                          dtype=np.float32).reshape(1, C)


def build_in_maps(x, embed, adj, noise, tmp, W1, b1, W2, b2, Wg1, Wg2):
    embT = np.ascontiguousarray(embed.T)
    in_maps = []
    for c in range(NC):
        sl = slice(c * R, (c + 1) * R)
        in_maps.append({
            "embT_in": embT,
            "embTs_in": np.ascontiguousarray(embT[:, sl]),
            "x_full": x,
            "adjcol_in": np.ascontiguousarray(adj[sl].T),
            "noise_slab": np.ascontiguousarray(noise[sl]),
            "tmp_in": tmp,
            "w1_in": W1,
            "b1_in": b1,
            "w2_in": W2,
            "b2_in": b2,
            "wg1_in": Wg1,
            "wg2_in": Wg2,
        })
    return in_maps


# revision 10
# speedup vs baseline: 1.1217x; 1.1217x over previous
"""Trainium2 Bass kernel for nn_Explainer (gnn_message_passing).

Math (reference):
  f12[i*n+j] = concat(embed[i], embed[j]);  h = relu(f12 @ W1 + b1)
  log_alpha = h @ W2 + b2
  gate = sigmoid((log(u) - log(1-u) + log_alpha) / beta)
  sym = (gate + gate.T)/2 ; masked = adj * sym
  hg = relu((masked @ x) @ Wg1); pooled = hg.mean(0); softmax(pooled @ Wg2)

Key decomposition: f12 @ W1 + b1 = A[i] + B[j] with
  A = embed @ W1[:64] + b1   (per-row), B = embed @ W1[64:]
so log_alpha[i,j] = W2 . relu(A[i] + B[j]) + b2 -- no [N^2,128] matmul needed.

Sharding: row-blocks of the i dimension across 8 cores. The pre-sigmoid
log-odds (pre = nl + la) are exchanged with ONE AllToAll in bf16; both
sides apply the sigmoid locally (it commutes with the exchange), keeping
the staging path off the ACT engine. Per-core class logits are combined
with a tiny AllToAll (cheaper than AllReduce at this size) + ones-matmul.

The CC engine has a fixed ~43us init (the profile's leading BARRIER ends
~64us local regardless of program), so everything before ~60us is free;
the design minimizes the post-barrier serial chain: A2A -> scatter ->
sigmoid -> mask -> GNN -> logits -> logit-A2A -> softmax.
"""
import numpy as np

import concourse.bass as bass
import concourse.bacc as bacc
import concourse.tile as tile
from concourse import mybir
from concourse.bass_utils import run_bass_kernel_spmd

N = 1024
NC = 8
R = N // NC          # 128 rows per core
D = 64               # embed dim
H = 64               # hidden
F = 128              # x features
C = 8                # classes
NPAIR = R // 2       # 64 i-pairs per core
GRP = 16             # pairs per PE column-group / psum row-group
NCHUNK = NPAIR // GRP  # 4 row-chunks of 32 pre rows
CH = 2 * GRP         # 32 rows per chunk

F32 = mybir.dt.float32
BF16 = mybir.dt.bfloat16
MM_DT = BF16
DEBUG_OUTPUTS = False


def _mask_w2_np():
    """[128, NPAIR, 32] mask: 1.0 where the block-diag W2 stack has W2 values.

    Pair t -> psum row-group g=t//16 (tile_position=(0,32g)), slot s=t%16.
    lhsT_t = W2S[:, t, :]: col 2s rows 0:64 = W2, col 2s+1 rows 64:128 = W2.
    psum out row for pair t = 32g + 2s (+1) = 2t (+1) = local i'.
    """
    cols = 32
    m = np.zeros((128, NPAIR, cols), np.float32)
    for t in range(NPAIR):
        s = t % GRP
        m[0:64, t, 2 * s] = 1.0
        m[64:128, t, 2 * s + 1] = 1.0
    return m


def build():
    nc = bacc.Bacc("TRN2", target_bir_lowering=False, debug=False, num_devices=NC)

    # ---- kernel I/O (all fat row-major layouts; transposes happen on PE) ----
    embT_in = nc.dram_tensor("embT_in", [D, N], F32, kind="ExternalInput")
    embTs_in = nc.dram_tensor("embTs_in", [D, R], F32, kind="ExternalInput")
    xT_in = nc.dram_tensor("xT_in", [F, N], F32, kind="ExternalInput")
    adjrow_in = nc.dram_tensor("adjrow_in", [R, N], F32, kind="ExternalInput")
    noise_slab = nc.dram_tensor("noise_slab", [R, N], F32, kind="ExternalInput")
    tmp_in = nc.dram_tensor("tmp_in", [1, 1], F32, kind="ExternalInput")
    w1_in = nc.dram_tensor("w1_in", [2 * D, H], F32, kind="ExternalInput")
    b1_in = nc.dram_tensor("b1_in", [1, H], F32, kind="ExternalInput")
    w2_in = nc.dram_tensor("w2_in", [H, 1], F32, kind="ExternalInput")
    b2_in = nc.dram_tensor("b2_in", [1, 1], F32, kind="ExternalInput")
    wg1_in = nc.dram_tensor("wg1_in", [F, H], F32, kind="ExternalInput")
    wg2_in = nc.dram_tensor("wg2_in", [H, C], F32, kind="ExternalInput")
    out_dram = nc.dram_tensor("out", [1, C], F32, kind="ExternalOutput")
    dbg = {}
    if DEBUG_OUTPUTS:
        for nm, shp in [("d_pre", [R, N]), ("d_gate", [R, N]),
                        ("d_mh", [128, N]), ("d_tT", [128, 128]),
                        ("d_pooled", [1, H]), ("d_lgp", [1, C])]:
            dbg[nm] = nc.dram_tensor(nm, shp, F32, kind="ExternalOutput")

    # ---- compile-time constants ----
    identb_c = nc.inline_tensor(np.eye(128, dtype=np.float32).astype(
        mybir.dt.np(BF16)), name="identb")
    identf_c = nc.inline_tensor(np.eye(128, dtype=np.float32), name="identf")
    maskw2_c = nc.inline_tensor(_mask_w2_np().astype(
        np.float32).astype(mybir.dt.np(MM_DT)), name="maskw2")
    ones128_c = nc.inline_tensor(np.ones((1, 128), np.float32), name="ones128")
    ones8_c = nc.inline_tensor(np.ones((8, 1), np.float32), name="ones8")

    with tile.TileContext(nc) as tc:
        with (
            tc.tile_pool(name="const", bufs=1) as constp,
            tc.tile_pool(name="big", bufs=1) as big,
            tc.tile_pool(name="tmpp", bufs=6) as tmpp,
            tc.tile_pool(name="pla", bufs=1, space="PSUM") as pla,
            tc.tile_pool(name="ptp", bufs=2, space="PSUM") as ptp,
            tc.tile_pool(name="psm", bufs=2, space="PSUM") as psm,
            tc.tile_pool(name="dram", bufs=1, space="DRAM") as dram,
        ):
            # ============ phase 0: loads + precompute =======================
            # critical-path loads on the sync (SP) queue; its dispatch does
            # not steal DVE/ACT time.
            embT = big.tile([D, N], F32)
            nc.sync.dma_start(embT[:], embT_in[:])
            w1a_sb = big.tile([D, H], F32)
            nc.sync.dma_start(w1a_sb[:], w1_in[0:D, :])
            w1b_sb = big.tile([D, H], F32)
            nc.sync.dma_start(w1b_sb[:], w1_in[D:2 * D, :])
            eTs = big.tile([D, R], F32)
            nc.sync.dma_start(eTs[:], embTs_in[:])
            b1t_sb = big.tile([H, 1], F32)
            nc.sync.dma_start(b1t_sb[:], b1_in[:].rearrange("o h -> h o"))
            w2_sb = big.tile([H, 1], F32)
            nc.sync.dma_start(w2_sb[:], w2_in[:])
            noise_sb = big.tile([R, N], F32)
            nc.sync.dma_start(noise_sb[:], noise_slab[:])
            maskw2 = constp.tile([128, NPAIR, 32], MM_DT)
            nc.sync.dma_start(maskw2[:, 0:GRP, :], maskw2_c[:, 0:GRP, :])
            nc.sync.dma_start(maskw2[:, GRP:NPAIR, :],
                              maskw2_c[:, GRP:NPAIR, :])
            xT_sb = big.tile([F, N], F32)
            nc.sync.dma_start(xT_sb[:], xT_in[:])
            adjrow = big.tile([R, N], F32)
            nc.sync.dma_start(adjrow[:], adjrow_in[:])

            # small loads on gpsimd (cheap dispatch, engine mostly idle)
            identb = constp.tile([128, 128], BF16)
            nc.gpsimd.dma_start(identb[:], identb_c[:])
            identf = constp.tile([128, 128], F32)
            nc.gpsimd.dma_start(identf[:], identf_c[:])
            ones128 = constp.tile([1, 128], F32)
            nc.gpsimd.dma_start(ones128[:], ones128_c[:])
            ones8 = constp.tile([8, 1], F32)
            nc.gpsimd.dma_start(ones8[:], ones8_c[:])
            b2_sb = big.tile([1, 1], F32)
            nc.gpsimd.dma_start(b2_sb[:], b2_in[:])
            tmp_sb = big.tile([1, 1], F32)
            nc.gpsimd.dma_start(tmp_sb[:], tmp_in[:])
            wg1_sb = big.tile([F, H], F32)
            nc.gpsimd.dma_start(wg1_sb[:], wg1_in[:])
            wg2_sb = big.tile([H, C], F32)
            nc.gpsimd.dma_start(wg2_sb[:], wg2_in[:])

            # PE warm-up: dependency-free dummy matmuls so the HAM clock-gate
            # opens (1.2 -> 2.4 GHz) before the real matmul stream starts.
            warm_sb = tmpp.tile([128, 512], MM_DT, tag="warm")
            nc.vector.memset(warm_sb[:], 0.0)
            for _ in range(12):
                warm_ps = psm.tile([1, 512], F32, tag="sm", name="warm_ps")
                nc.tensor.matmul(warm_ps[:], warm_sb[:, 0:1], warm_sb[:])

            # scaled GNN weights: 0.5 into Wg1 (symmetrize), 1/1024 into Wg2
            wg1h = big.tile([F, H], BF16)
            nc.gpsimd.tensor_scalar(out=wg1h[:], in0=wg1_sb[:],
                                    scalar1=0.5, scalar2=None,
                                    op0=mybir.AluOpType.mult)
            wg2s = big.tile([H, C], F32)
            nc.gpsimd.tensor_scalar(out=wg2s[:], in0=wg2_sb[:],
                                    scalar1=1.0 / N, scalar2=None,
                                    op0=mybir.AluOpType.mult)

            # A^T for this core's slab: [64, 128] = W1a^T @ embed_slab^T + b1
            at_ps = psm.tile([H, R], F32, tag="sm")
            nc.tensor.matmul(at_ps[:], w1a_sb[:], eTs[:])
            ats = big.tile([H, R], F32)
            nc.vector.tensor_scalar(out=ats[:], in0=at_ps[:],
                                    scalar1=b1t_sb[:], scalar2=None,
                                    op0=mybir.AluOpType.add)
            atstack = big.tile([128, NPAIR], F32)
            ats_pair = ats[:].rearrange("h (t two) -> h two t", two=2)
            nc.vector.tensor_copy(atstack[0:H, :], ats_pair[:, 0, :])
            nc.vector.tensor_copy(atstack[H:128, :], ats_pair[:, 1, :])

            # B^T (full): [64, 1024], then stacked twice -> [128, 1024] bf16
            btstack = big.tile([128, N], MM_DT)
            for jc in range(2):
                bt_ps = psm.tile([H, 512], F32, tag="sm")
                nc.tensor.matmul(bt_ps[:], w1b_sb[:],
                                 embT[:, jc * 512:(jc + 1) * 512])
                nc.vector.tensor_copy(
                    btstack[0:H, jc * 512:(jc + 1) * 512], bt_ps[:])
                nc.scalar.copy(
                    btstack[H:128, jc * 512:(jc + 1) * 512], bt_ps[:])

            # W2 stacks: maskw2 * [W2; W2] per-partition (group 0 first)
            w2col = big.tile([128, 1], F32)
            nc.vector.tensor_copy(w2col[0:H, :], w2_sb[:])
            nc.vector.tensor_copy(w2col[H:128, :], w2_sb[:])
            w2s_t = big.tile([128, NPAIR, 32], MM_DT)
            nc.vector.tensor_scalar(
                out=w2s_t[:, 0:GRP, :].rearrange("p t c -> p (t c)"),
                in0=maskw2[:, 0:GRP, :].rearrange("p t c -> p (t c)"),
                scalar1=w2col[:], scalar2=None, op0=mybir.AluOpType.mult)
            nc.vector.tensor_scalar(
                out=w2s_t[:, GRP:NPAIR, :].rearrange("p t c -> p (t c)"),
                in0=maskw2[:, GRP:NPAIR, :].rearrange("p t c -> p (t c)"),
                scalar1=w2col[:], scalar2=None, op0=mybir.AluOpType.mult)

            # gate scale/bias: sigmoid(invb * pre + invb*b2)
            invb = big.tile([1, 1], F32)
            nc.vector.reciprocal(invb[:], tmp_sb[:])
            ib2 = big.tile([1, 1], F32)
            nc.vector.tensor_tensor(ib2[:], invb[:], b2_sb[:],
                                    op=mybir.AluOpType.mult)
            invb_ps = psm.tile([128, 1], F32, tag="sm")
            nc.tensor.matmul(invb_ps[:], ones128[:], invb[:])
            invb128 = big.tile([128, 1], F32)
            nc.vector.tensor_copy(invb128[:], invb_ps[:])
            ib2_ps = psm.tile([128, 1], F32, tag="sm")
            nc.tensor.matmul(ib2_ps[:], ones128[:], ib2[:])
            ib2b = big.tile([128, 1], F32)
            nc.vector.tensor_copy(ib2b[:], ib2_ps[:])

            # noise transform on ACT: nl = ln(u) - ln(1-u)
            logu = big.tile([R, N], F32)
            nc.scalar.activation(logu[:], noise_sb[:],
                                 mybir.ActivationFunctionType.Ln)
            log1mu = big.tile([R, N], F32)
            nc.scalar.activation(log1mu[:], noise_sb[:],
                                 mybir.ActivationFunctionType.Ln,
                                 bias=1.0, scale=-1.0)
            nl = big.tile([R, N], F32)
            nc.vector.tensor_tensor(nl[:], logu[:], log1mu[:],
                                    op=mybir.AluOpType.subtract)

            # ========= phase 1: edge MLP, pre staged per psum group =========
            la_ps = [pla.tile([128, 512], F32, tag=f"la{jc}", name=f"la_ps{jc}")
                     for jc in range(2)]
            pre = big.tile([R, N], BF16)
            a2a_in = dram.tile([N, 128], BF16)
            a2a_in_r = a2a_in[:].rearrange("(r m) i -> m r i", r=NC)
            for t in range(NPAIR):
                g, s = t // GRP, t % GRP
                tmpb = tmpp.tile([128, N], MM_DT, tag="relu")
                if t % 5 == 2:
                    nc.scalar.activation(
                        tmpb[:], btstack[:],
                        mybir.ActivationFunctionType.Relu,
                        bias=atstack[:, t:t + 1])
                else:
                    nc.vector.tensor_scalar(
                        out=tmpb[:], in0=btstack[:],
                        scalar1=atstack[:, t:t + 1], scalar2=0.0,
                        op0=mybir.AluOpType.add, op1=mybir.AluOpType.max)
                for jc in range(2):
                    nc.tensor.matmul(
                        la_ps[jc][32 * g:32 * (g + 1), :],
                        w2s_t[:, t, :],
                        tmpb[:, jc * 512:(jc + 1) * 512],
                        start=(s == 0), stop=(s == GRP - 1),
                        tile_position=(0, 32 * g))
                if s == GRP - 1:
                    # group g's psum rows are final: pre chunk (bf16) + stage
                    lo, hi = 32 * g, 32 * (g + 1)
                    for jc in range(2):
                        nc.vector.tensor_tensor(
                            pre[lo:hi, jc * 512:(jc + 1) * 512],
                            la_ps[jc][lo:hi, :],
                            nl[lo:hi, jc * 512:(jc + 1) * 512],
                            op=mybir.AluOpType.add)
                    nc.sync.dma_start(
                        a2a_in_r[lo:hi],
                        pre[lo:hi, :].rearrange("m (r i) -> m r i", r=NC))

            # single AllToAll of the log-odds (doorbell rings ~45us, data
            # flows as soon as the CC init barrier ends)
            a2a_out = dram.tile([N, 128], BF16)
            nc.gpsimd.collective_compute(
                "AllToAll", mybir.AluOpType.bypass,
                replica_groups=[list(range(NC))],
                ins=[a2a_in[:].opt()], outs=[a2a_out[:].opt()])

            # x / adj transposed on PE into [j-partition] layouts while the
            # A2A is in flight (the DMA rearrange version costs 1024 skinny
            # descriptors; this is fat loads + 16 transposes). Emitted after
            # the pair loop so the blocked transposes never clog the PE
            # queue's 4-deep bypass window during phase 1.
            x_bf = big.tile([128, NC, F], BF16)
            adjT_bf = big.tile([128, NC, 128], BF16)
            for r in range(NC):
                px = psm.tile([128, 128], F32, tag="sm", name=f"px{r}")
                nc.tensor.transpose(px[:], xT_sb[:, r * 128:(r + 1) * 128],
                                    identf[:])
                nc.scalar.copy(x_bf[:, r, :], px[:])
            for r in range(NC):
                pa = psm.tile([128, 128], F32, tag="sm", name=f"pa{r}")
                nc.tensor.transpose(pa[:], adjrow[:, r * 128:(r + 1) * 128],
                                    identf[:])
                nc.scalar.copy(adjT_bf[:, r, :], pa[:])

            # own gate + transposes while the A2A is in flight
            gate = big.tile([R, N], BF16)
            nc.scalar.activation(gate[:], pre[:],
                                 mybir.ActivationFunctionType.Sigmoid,
                                 bias=ib2b[:], scale=invb128[:])
            gTc = big.tile([128, NC, 128], BF16)
            for r in range(NC):
                pt = ptp.tile([128, 128], BF16, tag="tpb")
                nc.tensor.transpose(pt[:], gate[:, r * 128:(r + 1) * 128],
                                    identb[:])
                nc.vector.tensor_copy(gTc[:, r, :], pt[:])

            # scatter received pre columns (split across 2 DMA queues),
            # then sigmoid them into gcolT
            gcolP = big.tile([128, NC, 128], BF16)
            a2a_out_r = a2a_out[:].rearrange("(r m) i -> m r i", r=NC)
            nc.sync.dma_start(gcolP[:, 0:4, :], a2a_out_r[:, 0:4, :])
            nc.gpsimd.dma_start(gcolP[:, 4:8, :], a2a_out_r[:, 4:8, :])
            gcolT = big.tile([128, NC, 128], BF16)
            nc.scalar.activation(gcolT[:].rearrange("p r b -> p (r b)"),
                                 gcolP[:].rearrange("p r b -> p (r b)"),
                                 mybir.ActivationFunctionType.Sigmoid,
                                 bias=ib2b[:], scale=invb128[:])

            # ================= phase 3: mask + GNN ==========================
            msum = big.tile([128, N], BF16)
            nc.vector.tensor_tensor(msum[:],
                                    gTc[:].rearrange("p r b -> p (r b)"),
                                    gcolT[:].rearrange("p r b -> p (r b)"),
                                    op=mybir.AluOpType.add)
            mh_bf = big.tile([128, N], BF16)
            nc.vector.tensor_tensor(mh_bf[:], msum[:],
                                    adjT_bf[:].rearrange("p r b -> p (r b)"),
                                    op=mybir.AluOpType.mult)

            # tT[f, i'] = sum_j x[j, f] masked[i', j]
            tT_ps = pla.tile([128, 128], F32, tag="tT")
            for r in range(NC):
                nc.tensor.matmul(
                    tT_ps[:], x_bf[:, r, :],
                    mh_bf[:, r * 128:(r + 1) * 128],
                    start=(r == 0), stop=(r == NC - 1))
            tT = big.tile([128, 128], BF16)
            nc.vector.tensor_copy(tT[:], tT_ps[:])

            # hgT = relu(Wg1h^T @ tT): [64, 128]; pooled partial via ACT accum
            hg_ps = psm.tile([H, 128], F32, tag="sm")
            nc.tensor.matmul(hg_ps[:], wg1h[:], tT[:])
            hgT = big.tile([H, 128], F32)
            pooled = big.tile([H, 1], F32)
            nc.scalar.activation(hgT[:], hg_ps[:],
                                 mybir.ActivationFunctionType.Relu,
                                 accum_out=pooled[:])

            # local class logits; broadcast to [8, C] rows for the logit A2A
            lg_ps = psm.tile([1, C], F32, tag="sm")
            nc.tensor.matmul(lg_ps[:], pooled[:], wg2s[:])
            lgp = big.tile([1, C], F32)
            nc.vector.tensor_copy(lgp[:], lg_ps[:])
            lg8_ps = psm.tile([NC, C], F32, tag="sm")
            nc.tensor.matmul(lg8_ps[:], ones128[:, 0:NC], lgp[:])
            lg8 = big.tile([NC, C], F32)
            nc.vector.tensor_copy(lg8[:], lg8_ps[:])

            # ACT exp-table preload, gated on lgp so it runs during the
            # logit exchange (relu lives in every table; no reload later)
            dexp = big.tile([1, 8], F32)
            nc.scalar.activation(dexp[:], lgp[:],
                                 mybir.ActivationFunctionType.Exp)

            # tiny AllToAll = allgather of per-core logits ([1,C] per rank)
            lga_in = dram.tile([NC, C], F32)
            nc.gpsimd.dma_start(lga_in[:], lg8[:])
            lga_out = dram.tile([NC, C], F32)
            nc.gpsimd.collective_compute(
                "AllToAll", mybir.AluOpType.bypass,
                replica_groups=[list(range(NC))],
                ins=[lga_in[:].opt()], outs=[lga_out[:].opt()])
            z8 = big.tile([NC, C], F32)
            nc.sync.dma_start(z8[:], lga_out[:])
            z_ps = psm.tile([1, C], F32, tag="sm")
            nc.tensor.matmul(z_ps[:], ones8[:], z8[:])
            z = big.tile([1, C], F32)
            nc.vector.tensor_copy(z[:], z_ps[:])

            # softmax on [1, 8] (logits are O(1): skip the max-subtraction)
            e = big.tile([1, C], F32)
            ssum = big.tile([1, 1], F32)
            nc.scalar.activation(e[:], z[:],
                                 mybir.ActivationFunctionType.Exp,
                                 accum_out=ssum[:])
            rinv = big.tile([1, 1], F32)
            nc.vector.reciprocal(rinv[:], ssum[:])
            sm = big.tile([1, C], F32)
            nc.vector.tensor_scalar(out=sm[:], in0=e[:], scalar1=rinv[:],
                                    scalar2=None, op0=mybir.AluOpType.mult)
            nc.sync.dma_start(out_dram[:], sm[:])

            if DEBUG_OUTPUTS:
                pf = big.tile([R, N], F32)
                nc.vector.tensor_copy(pf[:], pre[:])
                nc.sync.dma_start(dbg["d_pre"][:], pf[:])
                gf = big.tile([R, N], F32)
                nc.vector.tensor_copy(gf[:], gate[:])
                nc.sync.dma_start(dbg["d_gate"][:], gf[:])
                mf = big.tile([128, N], F32)
                nc.vector.tensor_copy(mf[:], mh_bf[:])
                nc.sync.dma_start(dbg["d_mh"][:], mf[:])
                tf = big.tile([128, 128], F32)
                nc.vector.tensor_copy(tf[:], tT[:])
                nc.sync.dma_start(dbg["d_tT"][:], tf[:])
                nc.sync.dma_start(dbg["d_pooled"][:].rearrange("o h -> h o"),
                                  pooled[:])
                nc.sync.dma_start(dbg["d_lgp"][:], lgp[:])

    nc.compile()
    return nc


_NC_CACHE = None
_RUNNER_CACHE = None


def _get_nc():
    global _NC_CACHE
    if _NC_CACHE is None:
        _NC_CACHE = build()
    return _NC_CACHE


def _get_runner():
    """Cached jitted 8-core executable (run_bass_via_pjrt rebuilds the jit
    wrapper every call, costing ~300ms of host time per invocation)."""
    global _RUNNER_CACHE
    if _RUNNER_CACHE is not None:
        return _RUNNER_CACHE
    import jax
    from jax.sharding import Mesh, PartitionSpec
    from jax.experimental.shard_map import shard_map
    from concourse import mybir as mb
    from concourse.bass2jax import (_bass_exec_p, install_neuronx_cc_hook,
                                    partition_id_tensor)

    nc = _get_nc()
    install_neuronx_cc_hook()
    partition_name = (nc.partition_id_tensor.name
                      if nc.partition_id_tensor else None)
    in_names, out_names, out_avals, zero_outs = [], [], [], []
    for alloc in nc.m.functions[0].allocations:
        if not isinstance(alloc, mb.MemoryLocationSet):
            continue
        name = alloc.memorylocations[0].name
        if alloc.kind == "ExternalInput":
            if name == partition_name:
                continue
            in_names.append(name)
        elif alloc.kind == "ExternalOutput":
            shape = tuple(alloc.tensor_shape)
            dtype = mb.dt.np(alloc.dtype)
            out_names.append(name)
            out_avals.append(jax.core.ShapedArray(shape, dtype))
            zero_outs.append(np.zeros(shape, dtype))
    n_params = len(in_names)
    all_in = in_names + out_names
    if partition_name is not None:
        all_in = all_in + [partition_name]

    def _body(*args):
        operands = list(args)
        if partition_name is not None:
            operands.append(partition_id_tensor())
        outs = _bass_exec_p.bind(
            *operands,
            out_avals=tuple(out_avals),
            in_names=tuple(all_in),
            out_names=tuple(out_names),
            lowering_input_output_aliases=(),
            sim_require_finite=True,
            sim_require_nnan=True,
            nc=nc,
        )
        return tuple(outs)

    devices = jax.devices()[:NC]
    mesh = Mesh(np.asarray(devices), ("core",))
    n_outs = len(out_names)
    sharded = jax.jit(
        shard_map(_body, mesh=mesh,
                  in_specs=(PartitionSpec("core"),) * (n_params + n_outs),
                  out_specs=(PartitionSpec("core"),) * n_outs,
                  check_rep=False),
        donate_argnums=tuple(range(n_params, n_params + n_outs)),
        keep_unused=True)

    def run(in_maps):
        concat_in = [
            np.concatenate([np.asarray(in_maps[c][nm]) for c in range(NC)],
                           axis=0)
            for nm in in_names
        ]
        concat_zeros = [
            np.zeros((NC * z.shape[0], *z.shape[1:]), z.dtype)
            for z in zero_outs
        ]
        out_arrs = sharded(*concat_in, *concat_zeros)
        return [
            {nm: np.asarray(out_arrs[i]).reshape(NC, *out_avals[i].shape)[c]
             for i, nm in enumerate(out_names)}
            for c in range(NC)
        ]

    _RUNNER_CACHE = run
    return run


def kernel(**inputs):
    x = np.ascontiguousarray(np.asarray(inputs["x"], dtype=np.float32))
    embed = np.ascontiguousarray(np.asarray(inputs["embed"], dtype=np.float32))
    adj = np.ascontiguousarray(np.asarray(inputs["adj"], dtype=np.float32))
    tmp = np.asarray(inputs["tmp"], dtype=np.float32).reshape(1, 1)
    noise = np.asarray(inputs["noise"], dtype=np.float32).reshape(N, N)
    W1 = np.ascontiguousarray(np.asarray(inputs["W1"], dtype=np.float32))
    b1 = np.asarray(inputs["b1"], dtype=np.float32).reshape(1, H)
    W2 = np.ascontiguousarray(np.asarray(inputs["W2"], dtype=np.float32))
    b2 = np.asarray(inputs["b2"], dtype=np.float32).reshape(1, 1)
    Wg1 = np.ascontiguousarray(np.asarray(inputs["Wg1"], dtype=np.float32))
    Wg2 = np.ascontiguousarray(np.asarray(inputs["Wg2"], dtype=np.float32))

    in_maps = build_in_maps(x, embed, adj, noise, tmp, W1, b1, W2, b2, Wg1, Wg2)
    try:
        results = _get_runner()(in_maps)
        return np.asarray(results[0]["out"], dtype=np.float32).reshape(1, C)
    except Exception:
        nc = _get_nc()
        res = run_bass_kernel_spmd(nc, in_maps, core_ids=list(range(NC)))
        return np.asarray(res.results[0]["out"],
                          dtype=np.float32).reshape(1, C)


def build_in_maps(x, embed, adj, noise, tmp, W1, b1, W2, b2, Wg1, Wg2):
    embT = np.ascontiguousarray(embed.T)
    xT = np.ascontiguousarray(x.T)
    in_maps = []
    for c in range(NC):
        sl = slice(c * R, (c + 1) * R)
        in_maps.append({
            "embT_in": embT,
            "embTs_in": np.ascontiguousarray(embT[:, sl]),
            "xT_in": xT,
            "adjrow_in": np.ascontiguousarray(adj[sl]),
            "noise_slab": np.ascontiguousarray(noise[sl]),
            "tmp_in": tmp,
            "w1_in": W1,
            "b1_in": b1,
            "w2_in": W2,
            "b2_in": b2,
            "wg1_in": Wg1,
            "wg2_in": Wg2,
        })
    return in_maps


# revision 13
# speedup vs baseline: 1.2672x; 1.1297x over previous
"""Trainium2 Bass kernel for nn_Explainer (gnn_message_passing).

Math (reference):
  f12[i*n+j] = concat(embed[i], embed[j]);  h = relu(f12 @ W1 + b1)
  log_alpha = h @ W2 + b2
  gate = sigmoid((log(u) - log(1-u) + log_alpha) / beta)
  sym = (gate + gate.T)/2 ; masked = adj * sym
  hg = relu((masked @ x) @ Wg1); pooled = hg.mean(0); softmax(pooled @ Wg2)

Key decomposition: f12 @ W1 + b1 = A[i] + B[j] with
  A = embed @ W1[:64] + b1   (per-row), B = embed @ W1[64:]
so log_alpha[i,j] = W2 . relu(A[i] + B[j]) + b2 -- no [N^2,128] matmul needed.

Sharding: row-blocks of the i dimension across 8 cores. The pre-sigmoid
log-odds (pre = nl + la) are exchanged with ONE AllToAll in bf16; both
sides apply the sigmoid locally (it commutes with the exchange), keeping
the staging path off the ACT engine. Per-core class logits are combined
with a tiny AllToAll (cheaper than AllReduce at this size) + ones-matmul.

The CC engine has a fixed ~43us init (the profile's leading BARRIER ends
~64us local regardless of program), so everything before ~60us is free;
the design minimizes the post-barrier serial chain: A2A -> scatter ->
sigmoid -> mask -> GNN -> logits -> logit-A2A -> softmax.
"""
import numpy as np

import concourse.bass as bass
import concourse.bacc as bacc
import concourse.tile as tile
from concourse import mybir
from concourse.bass_utils import run_bass_kernel_spmd

N = 1024
NC = 8
R = N // NC          # 128 rows per core
D = 64               # embed dim
H = 64               # hidden
F = 128              # x features
C = 8                # classes
NPAIR = R // 2       # 64 i-pairs per core
GRP = 16             # pairs per PE column-group / psum row-group
NCHUNK = NPAIR // GRP  # 4 row-chunks of 32 pre rows
CH = 2 * GRP         # 32 rows per chunk

F32 = mybir.dt.float32
BF16 = mybir.dt.bfloat16
MM_DT = BF16
DEBUG_OUTPUTS = False


def _mask_w2_np():
    """[128, NPAIR, 32] mask: 1.0 where the block-diag W2 stack has W2 values.

    Pair t -> psum row-group g=t//16 (tile_position=(0,32g)), slot s=t%16.
    lhsT_t = W2S[:, t, :]: col 2s rows 0:64 = W2, col 2s+1 rows 64:128 = W2.
    psum out row for pair t = 32g + 2s (+1) = 2t (+1) = local i'.
    """
    cols = 32
    m = np.zeros((128, NPAIR, cols), np.float32)
    for t in range(NPAIR):
        s = t % GRP
        m[0:64, t, 2 * s] = 1.0
        m[64:128, t, 2 * s + 1] = 1.0
    return m


def build():
    nc = bacc.Bacc("TRN2", target_bir_lowering=False, debug=False, num_devices=NC)

    # ---- kernel I/O (all fat row-major layouts; transposes happen on PE) ----
    embT_in = nc.dram_tensor("embT_in", [D, N], F32, kind="ExternalInput")
    embTs_in = nc.dram_tensor("embTs_in", [D, R], F32, kind="ExternalInput")
    xT_in = nc.dram_tensor("xT_in", [F, N], F32, kind="ExternalInput")
    adjrow_in = nc.dram_tensor("adjrow_in", [R, N], F32, kind="ExternalInput")
    noise_slab = nc.dram_tensor("noise_slab", [R, N], F32, kind="ExternalInput")
    tmp_in = nc.dram_tensor("tmp_in", [1, 1], F32, kind="ExternalInput")
    w1_in = nc.dram_tensor("w1_in", [2 * D, H], F32, kind="ExternalInput")
    b1_in = nc.dram_tensor("b1_in", [1, H], F32, kind="ExternalInput")
    w2_in = nc.dram_tensor("w2_in", [H, 1], F32, kind="ExternalInput")
    b2_in = nc.dram_tensor("b2_in", [1, 1], F32, kind="ExternalInput")
    wg1_in = nc.dram_tensor("wg1_in", [F, H], F32, kind="ExternalInput")
    wg2_in = nc.dram_tensor("wg2_in", [H, C], F32, kind="ExternalInput")
    out_dram = nc.dram_tensor("out", [1, C], F32, kind="ExternalOutput")
    dbg = {}
    if DEBUG_OUTPUTS:
        for nm, shp in [("d_pre", [R, N]), ("d_gate", [R, N]),
                        ("d_mh", [128, N]), ("d_tT", [128, 128]),
                        ("d_pooled", [1, H]), ("d_lgp", [1, C])]:
            dbg[nm] = nc.dram_tensor(nm, shp, F32, kind="ExternalOutput")

    # ---- compile-time constants ----
    identb_c = nc.inline_tensor(np.eye(128, dtype=np.float32).astype(
        mybir.dt.np(BF16)), name="identb")
    identf_c = nc.inline_tensor(np.eye(128, dtype=np.float32), name="identf")
    maskw2_c = nc.inline_tensor(_mask_w2_np().astype(
        np.float32).astype(mybir.dt.np(MM_DT)), name="maskw2")
    ones128_c = nc.inline_tensor(np.ones((1, 128), np.float32), name="ones128")
    ones8_c = nc.inline_tensor(np.ones((8, 1), np.float32), name="ones8")

    with tile.TileContext(nc) as tc:
        with (
            tc.tile_pool(name="const", bufs=1) as constp,
            tc.tile_pool(name="big", bufs=1) as big,
            tc.tile_pool(name="tmpp", bufs=6) as tmpp,
            tc.tile_pool(name="pla", bufs=1, space="PSUM") as pla,
            tc.tile_pool(name="ptp", bufs=2, space="PSUM") as ptp,
            tc.tile_pool(name="psm", bufs=2, space="PSUM") as psm,
            tc.tile_pool(name="dram", bufs=1, space="DRAM") as dram,
        ):
            # ============ phase 0: loads + precompute =======================
            # critical-path loads on the sync (SP) queue; its dispatch does
            # not steal DVE/ACT time.
            embT = big.tile([D, N], F32)
            nc.sync.dma_start(embT[:], embT_in[:])
            w1a_sb = big.tile([D, H], F32)
            nc.sync.dma_start(w1a_sb[:], w1_in[0:D, :])
            w1b_sb = big.tile([D, H], F32)
            nc.sync.dma_start(w1b_sb[:], w1_in[D:2 * D, :])
            eTs = big.tile([D, R], F32)
            nc.sync.dma_start(eTs[:], embTs_in[:])
            b1t_sb = big.tile([H, 1], F32)
            nc.sync.dma_start(b1t_sb[:], b1_in[:].rearrange("o h -> h o"))
            w2_sb = big.tile([H, 1], F32)
            nc.sync.dma_start(w2_sb[:], w2_in[:])
            noise_sb = big.tile([R, N], F32)
            nc.sync.dma_start(noise_sb[:], noise_slab[:])
            maskw2 = constp.tile([128, NPAIR, 32], MM_DT)
            nc.sync.dma_start(maskw2[:, 0:GRP, :], maskw2_c[:, 0:GRP, :])
            nc.sync.dma_start(maskw2[:, GRP:NPAIR, :],
                              maskw2_c[:, GRP:NPAIR, :])
            xT_sb = big.tile([F, N], F32)
            nc.sync.dma_start(xT_sb[:], xT_in[:])
            adjrow = big.tile([R, N], F32)
            nc.sync.dma_start(adjrow[:], adjrow_in[:])

            # small loads on gpsimd (cheap dispatch, engine mostly idle)
            identb = constp.tile([128, 128], BF16)
            nc.gpsimd.dma_start(identb[:], identb_c[:])
            identf = constp.tile([128, 128], F32)
            nc.gpsimd.dma_start(identf[:], identf_c[:])
            ones128 = constp.tile([1, 128], F32)
            nc.gpsimd.dma_start(ones128[:], ones128_c[:])
            ones8 = constp.tile([8, 1], F32)
            nc.gpsimd.dma_start(ones8[:], ones8_c[:])
            b2_sb = big.tile([1, 1], F32)
            nc.gpsimd.dma_start(b2_sb[:], b2_in[:])
            tmp_sb = big.tile([1, 1], F32)
            nc.gpsimd.dma_start(tmp_sb[:], tmp_in[:])
            wg1_sb = big.tile([F, H], F32)
            nc.gpsimd.dma_start(wg1_sb[:], wg1_in[:])
            wg2_sb = big.tile([H, C], F32)
            nc.gpsimd.dma_start(wg2_sb[:], wg2_in[:])

            # PE warm-up: dependency-free dummy matmuls so the HAM clock-gate
            # opens (1.2 -> 2.4 GHz) before the real matmul stream starts.
            warm_sb = tmpp.tile([128, 512], MM_DT, tag="warm")
            nc.vector.memset(warm_sb[:], 0.0)
            for _ in range(12):
                warm_ps = psm.tile([1, 512], F32, tag="sm", name="warm_ps")
                nc.tensor.matmul(warm_ps[:], warm_sb[:, 0:1], warm_sb[:])

            # scaled GNN weights: 0.5 into Wg1 (symmetrize), 1/1024 into Wg2
            wg1h = big.tile([F, H], BF16)
            nc.gpsimd.tensor_scalar(out=wg1h[:], in0=wg1_sb[:],
                                    scalar1=0.5, scalar2=None,
                                    op0=mybir.AluOpType.mult)
            wg2s = big.tile([H, C], F32)
            nc.gpsimd.tensor_scalar(out=wg2s[:], in0=wg2_sb[:],
                                    scalar1=1.0 / N, scalar2=None,
                                    op0=mybir.AluOpType.mult)

            # A^T for this core's slab: [64, 128] = W1a^T @ embed_slab^T + b1
            at_ps = psm.tile([H, R], F32, tag="sm")
            nc.tensor.matmul(at_ps[:], w1a_sb[:], eTs[:])
            ats = big.tile([H, R], F32)
            nc.vector.tensor_scalar(out=ats[:], in0=at_ps[:],
                                    scalar1=b1t_sb[:], scalar2=None,
                                    op0=mybir.AluOpType.add)
            atstack = big.tile([128, NPAIR], F32)
            ats_pair = ats[:].rearrange("h (t two) -> h two t", two=2)
            nc.vector.tensor_copy(atstack[0:H, :], ats_pair[:, 0, :])
            nc.vector.tensor_copy(atstack[H:128, :], ats_pair[:, 1, :])

            # B^T (full): [64, 1024], then stacked twice -> [128, 1024] bf16
            btstack = big.tile([128, N], MM_DT)
            for jc in range(2):
                bt_ps = psm.tile([H, 512], F32, tag="sm")
                nc.tensor.matmul(bt_ps[:], w1b_sb[:],
                                 embT[:, jc * 512:(jc + 1) * 512])
                nc.vector.tensor_copy(
                    btstack[0:H, jc * 512:(jc + 1) * 512], bt_ps[:])
                nc.scalar.copy(
                    btstack[H:128, jc * 512:(jc + 1) * 512], bt_ps[:])

            # W2 stacks: maskw2 * [W2; W2] per-partition (group 0 first)
            w2col = big.tile([128, 1], F32)
            nc.vector.tensor_copy(w2col[0:H, :], w2_sb[:])
            nc.vector.tensor_copy(w2col[H:128, :], w2_sb[:])
            w2s_t = big.tile([128, NPAIR, 32], MM_DT)
            nc.vector.tensor_scalar(
                out=w2s_t[:, 0:GRP, :].rearrange("p t c -> p (t c)"),
                in0=maskw2[:, 0:GRP, :].rearrange("p t c -> p (t c)"),
                scalar1=w2col[:], scalar2=None, op0=mybir.AluOpType.mult)
            nc.vector.tensor_scalar(
                out=w2s_t[:, GRP:NPAIR, :].rearrange("p t c -> p (t c)"),
                in0=maskw2[:, GRP:NPAIR, :].rearrange("p t c -> p (t c)"),
                scalar1=w2col[:], scalar2=None, op0=mybir.AluOpType.mult)

            # gate scale/bias: sigmoid(invb * pre + invb*b2)
            invb = big.tile([1, 1], F32)
            nc.vector.reciprocal(invb[:], tmp_sb[:])
            ib2 = big.tile([1, 1], F32)
            nc.vector.tensor_tensor(ib2[:], invb[:], b2_sb[:],
                                    op=mybir.AluOpType.mult)
            invb_ps = psm.tile([128, 1], F32, tag="sm")
            nc.tensor.matmul(invb_ps[:], ones128[:], invb[:])
            invb128 = big.tile([128, 1], F32)
            nc.vector.tensor_copy(invb128[:], invb_ps[:])
            ib2_ps = psm.tile([128, 1], F32, tag="sm")
            nc.tensor.matmul(ib2_ps[:], ones128[:], ib2[:])
            ib2b = big.tile([128, 1], F32)
            nc.vector.tensor_copy(ib2b[:], ib2_ps[:])

            # noise transform on ACT: nl = ln(u) - ln(1-u)
            logu = big.tile([R, N], F32)
            nc.scalar.activation(logu[:], noise_sb[:],
                                 mybir.ActivationFunctionType.Ln)
            log1mu = big.tile([R, N], F32)
            nc.scalar.activation(log1mu[:], noise_sb[:],
                                 mybir.ActivationFunctionType.Ln,
                                 bias=1.0, scale=-1.0)
            nl = big.tile([R, N], F32)
            nc.vector.tensor_tensor(nl[:], logu[:], log1mu[:],
                                    op=mybir.AluOpType.subtract)

            # ========= phase 1: edge MLP, pre staged per psum group =========
            la_ps = [pla.tile([128, 512], F32, tag=f"la{jc}", name=f"la_ps{jc}")
                     for jc in range(2)]
            pre = big.tile([R, N], BF16)
            a2a_in = dram.tile([N, 128], BF16)
            a2a_in_r = a2a_in[:].rearrange("(r m) i -> m r i", r=NC)
            for t in range(NPAIR):
                g, s = t // GRP, t % GRP
                tmpb = tmpp.tile([128, N], MM_DT, tag="relu")
                if t % 5 == 2:
                    nc.scalar.activation(
                        tmpb[:], btstack[:],
                        mybir.ActivationFunctionType.Relu,
                        bias=atstack[:, t:t + 1])
                else:
                    nc.vector.tensor_scalar(
                        out=tmpb[:], in0=btstack[:],
                        scalar1=atstack[:, t:t + 1], scalar2=0.0,
                        op0=mybir.AluOpType.add, op1=mybir.AluOpType.max)
                for jc in range(2):
                    nc.tensor.matmul(
                        la_ps[jc][32 * g:32 * (g + 1), :],
                        w2s_t[:, t, :],
                        tmpb[:, jc * 512:(jc + 1) * 512],
                        start=(s == 0), stop=(s == GRP - 1),
                        tile_position=(0, 32 * g))
                if s == GRP - 1:
                    # group g's psum rows are final: pre chunk (bf16) + stage
                    lo, hi = 32 * g, 32 * (g + 1)
                    for jc in range(2):
                        nc.vector.tensor_tensor(
                            pre[lo:hi, jc * 512:(jc + 1) * 512],
                            la_ps[jc][lo:hi, :],
                            nl[lo:hi, jc * 512:(jc + 1) * 512],
                            op=mybir.AluOpType.add)
                    nc.sync.dma_start(
                        a2a_in_r[lo:hi],
                        pre[lo:hi, :].rearrange("m (r i) -> m r i", r=NC))

            # single AllToAll of the log-odds (doorbell rings ~45us, data
            # flows as soon as the CC init barrier ends). The payload is
            # bf16 but DECLARED f32 via bitcast: the CC engine's rate is
            # per-element (a bf16 A2A measured 2.2x slower than f32 for the
            # same element count), and bypass A2A is a pure byte move.
            a2a_out = dram.tile([N, 128], BF16)
            nc.gpsimd.collective_compute(
                "AllToAll", mybir.AluOpType.bypass,
                replica_groups=[list(range(NC))],
                ins=[a2a_in[:].bitcast(F32).opt()],
                outs=[a2a_out[:].bitcast(F32).opt()])

            # x / adj transposed on PE into [j-partition] layouts while the
            # A2A is in flight (the DMA rearrange version costs 1024 skinny
            # descriptors; this is fat loads + 16 transposes). Emitted after
            # the pair loop so the blocked transposes never clog the PE
            # queue's 4-deep bypass window during phase 1.
            x_bf = big.tile([128, NC, F], BF16)
            adjT_bf = big.tile([128, NC, 128], BF16)
            for r in range(NC):
                px = psm.tile([128, 128], F32, tag="sm", name=f"px{r}")
                nc.tensor.transpose(px[:], xT_sb[:, r * 128:(r + 1) * 128],
                                    identf[:])
                nc.scalar.copy(x_bf[:, r, :], px[:])
            for r in range(NC):
                pa = psm.tile([128, 128], F32, tag="sm", name=f"pa{r}")
                nc.tensor.transpose(pa[:], adjrow[:, r * 128:(r + 1) * 128],
                                    identf[:])
                nc.scalar.copy(adjT_bf[:, r, :], pa[:])

            # own gate + transposes while the A2A is in flight
            gate = big.tile([R, N], BF16)
            nc.scalar.activation(gate[:], pre[:],
                                 mybir.ActivationFunctionType.Sigmoid,
                                 bias=ib2b[:], scale=invb128[:])
            gTc = big.tile([128, NC, 128], BF16)
            for r in range(NC):
                pt = ptp.tile([128, 128], BF16, tag="tpb")
                nc.tensor.transpose(pt[:], gate[:, r * 128:(r + 1) * 128],
                                    identb[:])
                nc.vector.tensor_copy(gTc[:, r, :], pt[:])

            # scatter received pre columns (split across 4 DMA queues), then
            # sigmoid + mask in halves so each half overlaps the other's DMA
            gcolP = big.tile([128, NC, 128], BF16)
            a2a_out_r = a2a_out[:].rearrange("(r m) i -> m r i", r=NC)
            nc.sync.dma_start(gcolP[:, 0:4, :], a2a_out_r[:, 0:4, :])
            nc.gpsimd.dma_start(gcolP[:, 4:8, :], a2a_out_r[:, 4:8, :])
            gcolT = big.tile([128, NC, 128], BF16)
            msum = big.tile([128, N], BF16)
            mh_bf = big.tile([128, N], BF16)
            for h in range(2):
                lo, hi = 4 * h, 4 * (h + 1)
                nc.scalar.activation(
                    gcolT[:, lo:hi, :].rearrange("p r b -> p (r b)"),
                    gcolP[:, lo:hi, :].rearrange("p r b -> p (r b)"),
                    mybir.ActivationFunctionType.Sigmoid,
                    bias=ib2b[:], scale=invb128[:])
                nc.vector.tensor_tensor(
                    msum[:, lo * 128:hi * 128],
                    gTc[:, lo:hi, :].rearrange("p r b -> p (r b)"),
                    gcolT[:, lo:hi, :].rearrange("p r b -> p (r b)"),
                    op=mybir.AluOpType.add)
                nc.vector.tensor_tensor(
                    mh_bf[:, lo * 128:hi * 128], msum[:, lo * 128:hi * 128],
                    adjT_bf[:, lo:hi, :].rearrange("p r b -> p (r b)"),
                    op=mybir.AluOpType.mult)

            # tT[f, i'] = sum_j x[j, f] masked[i', j]
            tT_ps = pla.tile([128, 128], F32, tag="tT")
            for r in range(NC):
                nc.tensor.matmul(
                    tT_ps[:], x_bf[:, r, :],
                    mh_bf[:, r * 128:(r + 1) * 128],
                    start=(r == 0), stop=(r == NC - 1))
            tT = big.tile([128, 128], BF16)
            nc.vector.tensor_copy(tT[:], tT_ps[:])

            # hgT = relu(Wg1h^T @ tT): [64, 128]; pooled partial via ACT accum
            hg_ps = psm.tile([H, 128], F32, tag="sm")
            nc.tensor.matmul(hg_ps[:], wg1h[:], tT[:])
            hgT = big.tile([H, 128], F32)
            pooled = big.tile([H, 1], F32)
            nc.scalar.activation(hgT[:], hg_ps[:],
                                 mybir.ActivationFunctionType.Relu,
                                 accum_out=pooled[:])

            # local class logits; broadcast to [8, C] rows for the logit A2A
            lg_ps = psm.tile([1, C], F32, tag="sm")
            nc.tensor.matmul(lg_ps[:], pooled[:], wg2s[:])
            lgp = big.tile([1, C], F32)
            nc.vector.tensor_copy(lgp[:], lg_ps[:])
            lg8_ps = psm.tile([NC, C], F32, tag="sm")
            nc.tensor.matmul(lg8_ps[:], ones128[:, 0:NC], lgp[:])
            lg8 = big.tile([NC, C], F32)
            nc.vector.tensor_copy(lg8[:], lg8_ps[:])

            # ACT exp-table preload, gated on lgp so it runs during the
            # logit exchange (relu lives in every table; no reload later)
            dexp = big.tile([1, 8], F32)
            nc.scalar.activation(dexp[:], lgp[:],
                                 mybir.ActivationFunctionType.Exp)

            # tiny AllToAll = allgather of per-core logits ([1,C] per rank)
            lga_in = dram.tile([NC, C], F32)
            nc.gpsimd.dma_start(lga_in[:], lg8[:])
            lga_out = dram.tile([NC, C], F32)
            nc.gpsimd.collective_compute(
                "AllToAll", mybir.AluOpType.bypass,
                replica_groups=[list(range(NC))],
                ins=[lga_in[:].opt()], outs=[lga_out[:].opt()])
            z8 = big.tile([NC, C], F32)
            nc.sync.dma_start(z8[:], lga_out[:])
            z_ps = psm.tile([1, C], F32, tag="sm")
            nc.tensor.matmul(z_ps[:], ones8[:], z8[:])
            z = big.tile([1, C], F32)
            nc.vector.tensor_copy(z[:], z_ps[:])

            # softmax on [1, 8] (logits are O(1): skip the max-subtraction)
            e = big.tile([1, C], F32)
            ssum = big.tile([1, 1], F32)
            nc.scalar.activation(e[:], z[:],
                                 mybir.ActivationFunctionType.Exp,
                                 accum_out=ssum[:])
            rinv = big.tile([1, 1], F32)
            nc.vector.reciprocal(rinv[:], ssum[:])
            sm = big.tile([1, C], F32)
            nc.vector.tensor_scalar(out=sm[:], in0=e[:], scalar1=rinv[:],
                                    scalar2=None, op0=mybir.AluOpType.mult)
            nc.sync.dma_start(out_dram[:], sm[:])

            if DEBUG_OUTPUTS:
                pf = big.tile([R, N], F32)
                nc.vector.tensor_copy(pf[:], pre[:])
                nc.sync.dma_start(dbg["d_pre"][:], pf[:])
                gf = big.tile([R, N], F32)
                nc.vector.tensor_copy(gf[:], gate[:])
                nc.sync.dma_start(dbg["d_gate"][:], gf[:])
                mf = big.tile([128, N], F32)
                nc.vector.tensor_copy(mf[:], mh_bf[:])
                nc.sync.dma_start(dbg["d_mh"][:], mf[:])
                tf = big.tile([128, 128], F32)
                nc.vector.tensor_copy(tf[:], tT[:])
                nc.sync.dma_start(dbg["d_tT"][:], tf[:])
                nc.sync.dma_start(dbg["d_pooled"][:].rearrange("o h -> h o"),
                                  pooled[:])
                nc.sync.dma_start(dbg["d_lgp"][:], lgp[:])

    nc.compile()
    return nc


_NC_CACHE = None
_RUNNER_CACHE = None


def _get_nc():
    global _NC_CACHE
    if _NC_CACHE is None:
        _NC_CACHE = build()
    return _NC_CACHE


def _get_runner():
    """Cached jitted 8-core executable (run_bass_via_pjrt rebuilds the jit
    wrapper every call, costing ~300ms of host time per invocation)."""
    global _RUNNER_CACHE
    if _RUNNER_CACHE is not None:
        return _RUNNER_CACHE
    import jax
    from jax.sharding import Mesh, PartitionSpec
    from jax.experimental.shard_map import shard_map
    from concourse import mybir as mb
    from concourse.bass2jax import (_bass_exec_p, install_neuronx_cc_hook,
                                    partition_id_tensor)

    nc = _get_nc()
    install_neuronx_cc_hook()
    partition_name = (nc.partition_id_tensor.name
                      if nc.partition_id_tensor else None)
    in_names, out_names, out_avals, zero_outs = [], [], [], []
    for alloc in nc.m.functions[0].allocations:
        if not isinstance(alloc, mb.MemoryLocationSet):
            continue
        name = alloc.memorylocations[0].name
        if alloc.kind == "ExternalInput":
            if name == partition_name:
                continue
            in_names.append(name)
        elif alloc.kind == "ExternalOutput":
            shape = tuple(alloc.tensor_shape)
            dtype = mb.dt.np(alloc.dtype)
            out_names.append(name)
            out_avals.append(jax.core.ShapedArray(shape, dtype))
            zero_outs.append(np.zeros(shape, dtype))
    n_params = len(in_names)
    all_in = in_names + out_names
    if partition_name is not None:
        all_in = all_in + [partition_name]

    def _body(*args):
        operands = list(args)
        if partition_name is not None:
            operands.append(partition_id_tensor())
        outs = _bass_exec_p.bind(
            *operands,
            out_avals=tuple(out_avals),
            in_names=tuple(all_in),
            out_names=tuple(out_names),
            lowering_input_output_aliases=(),
            sim_require_finite=True,
            sim_require_nnan=True,
            nc=nc,
        )
        return tuple(outs)

    devices = jax.devices()[:NC]
    mesh = Mesh(np.asarray(devices), ("core",))
    n_outs = len(out_names)
    sharded = jax.jit(
        shard_map(_body, mesh=mesh,
                  in_specs=(PartitionSpec("core"),) * (n_params + n_outs),
                  out_specs=(PartitionSpec("core"),) * n_outs,
                  check_rep=False),
        donate_argnums=tuple(range(n_params, n_params + n_outs)),
        keep_unused=True)

    def run(in_maps):
        concat_in = [
            np.concatenate([np.asarray(in_maps[c][nm]) for c in range(NC)],
                           axis=0)
            for nm in in_names
        ]
        concat_zeros = [
            np.zeros((NC * z.shape[0], *z.shape[1:]), z.dtype)
            for z in zero_outs
        ]
        out_arrs = sharded(*concat_in, *concat_zeros)
        return [
            {nm: np.asarray(out_arrs[i]).reshape(NC, *out_avals[i].shape)[c]
             for i, nm in enumerate(out_names)}
            for c in range(NC)
        ]

    _RUNNER_CACHE = run
    return run


def kernel(**inputs):
    x = np.ascontiguousarray(np.asarray(inputs["x"], dtype=np.float32))
    embed = np.ascontiguousarray(np.asarray(inputs["embed"], dtype=np.float32))
    adj = np.ascontiguousarray(np.asarray(inputs["adj"], dtype=np.float32))
    tmp = np.asarray(inputs["tmp"], dtype=np.float32).reshape(1, 1)
    noise = np.asarray(inputs["noise"], dtype=np.float32).reshape(N, N)
    W1 = np.ascontiguousarray(np.asarray(inputs["W1"], dtype=np.float32))
    b1 = np.asarray(inputs["b1"], dtype=np.float32).reshape(1, H)
    W2 = np.ascontiguousarray(np.asarray(inputs["W2"], dtype=np.float32))
    b2 = np.asarray(inputs["b2"], dtype=np.float32).reshape(1, 1)
    Wg1 = np.ascontiguousarray(np.asarray(inputs["Wg1"], dtype=np.float32))
    Wg2 = np.ascontiguousarray(np.asarray(inputs["Wg2"], dtype=np.float32))

    in_maps = build_in_maps(x, embed, adj, noise, tmp, W1, b1, W2, b2, Wg1, Wg2)
    try:
        results = _get_runner()(in_maps)
        return np.asarray(results[0]["out"], dtype=np.float32).reshape(1, C)
    except Exception:
        nc = _get_nc()
        res = run_bass_kernel_spmd(nc, in_maps, core_ids=list(range(NC)))
        return np.asarray(res.results[0]["out"],
                          dtype=np.float32).reshape(1, C)


def build_in_maps(x, embed, adj, noise, tmp, W1, b1, W2, b2, Wg1, Wg2):
    embT = np.ascontiguousarray(embed.T)
    xT = np.ascontiguousarray(x.T)
    in_maps = []
    for c in range(NC):
        sl = slice(c * R, (c + 1) * R)
        in_maps.append({
            "embT_in": embT,
            "embTs_in": np.ascontiguousarray(embT[:, sl]),
            "xT_in": xT,
            "adjrow_in": np.ascontiguousarray(adj[sl]),
            "noise_slab": np.ascontiguousarray(noise[sl]),
            "tmp_in": tmp,
            "w1_in": W1,
            "b1_in": b1,
            "w2_in": W2,
            "b2_in": b2,
            "wg1_in": Wg1,
            "wg2_in": Wg2,
        })
    return in_maps
